# revision 35
# baseline (speedup 1.0000x reference)
"""BitMGQA forward on 8 trn2 NeuronCores, collective-free.

Core c owns batch b=c//4 and query rows (c%4)*512:(c%4+1)*512. Every core
recomputes the K/V projections for ALL 2048 keys of its batch (replicated
across the 4 cores of a batch group), so there is no cross-core
communication at all: no collectives, no rendezvous — each core's NEFF
runs to completion independently of the other cores' launch times.
Outputs are disjoint row slices -> host concat.

All matmuls are bf16 operands with f32 PSUM accumulation; LayerNorm
statistics and softmax normalization are computed in f32.
"""

import contextlib

import numpy as np

import concourse.bacc as bacc
import concourse.mybir as mybir
import concourse.tile as tile
from concourse.bass_utils import run_bass_kernel_spmd

B, T, C = 2, 2048, 2048
H, KV = 16, 4
HD = C // H  # 128
KVC = HD * KV  # 512
EPS = 1e-5
R = 512  # query rows per core
N_CORES = 8
SCALE = 1.0 / np.sqrt(HD)

F32 = mybir.dt.float32
F32R = mybir.dt.float32r
BF16 = mybir.dt.bfloat16
AF = mybir.ActivationFunctionType
ALU = mybir.AluOpType


def build_kernel(loop_n=1):
    nc = bacc.Bacc(
        "TRN2", target_bir_lowering=False, debug=False, num_devices=N_CORES
    )

    # Per-core inputs (host pre-transposed/tiled, see kernel() below)
    xq_d = nc.dram_tensor("xq", [128, 16, R], BF16, kind="ExternalInput").ap()
    # key/value of this core's batch, ^T tiled, chunk-major (chunk=512 keys)
    xk_d = nc.dram_tensor("xk", [4, 128, 16, 512], BF16, kind="ExternalInput").ap()
    xv_d = nc.dram_tensor("xv", [4, 128, 16, 512], BF16, kind="ExternalInput").ap()
    wq_d = nc.dram_tensor("wq", [16, 128, 16, 128], BF16, kind="ExternalInput").ap()
    wk_d = nc.dram_tensor("wk", [4, 128, 16, 128], BF16, kind="ExternalInput").ap()
    wv_d = nc.dram_tensor("wv", [128, 16, KVC], BF16, kind="ExternalInput").ap()
    wo_d = nc.dram_tensor("wo", [4, 128, 16, 512], BF16, kind="ExternalInput").ap()
    bq_d = nc.dram_tensor("bq", [128, 16], F32, kind="ExternalInput").ap()
    bk_d = nc.dram_tensor("bk", [128, 4], F32, kind="ExternalInput").ap()
    bv_d = nc.dram_tensor("bv", [1, KVC], F32R, kind="ExternalInput").ap()
    bo_d = nc.dram_tensor("bo", [1, C], F32R, kind="ExternalInput").ap()
    lnw_d = nc.dram_tensor("lnw", [128, 16], F32, kind="ExternalInput").ap()
    lnb_d = nc.dram_tensor("lnb", [128, 16], F32, kind="ExternalInput").ap()
    onesb_d = nc.dram_tensor("onesb", [128, 1], BF16, kind="ExternalInput").ap()
    onesr_d = nc.dram_tensor("onesr", [1, 512], F32R, kind="ExternalInput").ap()

    out_d = nc.dram_tensor("out", [R, C], F32, kind="ExternalOutput").ap()

    with tile.TileContext(nc) as tc:
        with (
            tc.tile_pool(name="consts", bufs=1) as consts,
            tc.tile_pool(name="wkp", bufs=4) as wkp,       # wk blocks [128,16,128]
            tc.tile_pool(name="big", bufs=2) as big,       # wv + wo blocks [128,16,512]
            tc.tile_pool(name="xs", bufs=2) as xs,         # x chunks [128,16,512]
            tc.tile_pool(name="wqp", bufs=2) as wqp,       # wq blocks [128,16,128]
            tc.tile_pool(name="qtb", bufs=16) as qtb_pool, # q^T per head [128,512]
            tc.tile_pool(name="ktf", bufs=4) as ktf_pool,  # k^T per group [128,2048]
            tc.tile_pool(name="vfb", bufs=16) as vfb_pool, # v tiles [128,512]
            tc.tile_pool(name="ytp", bufs=1) as ytp,       # y^T [128,16,512] bf16
            tc.tile_pool(name="att", bufs=17) as att_pool, # att tiles [128,512] bf16
            tc.tile_pool(name="blkf", bufs=4) as blkf,     # f32 scratch [128,512]
            tc.tile_pool(name="s1", bufs=4) as s1,         # [1,512] f32
            tc.tile_pool(name="ps", bufs=3, space="PSUM") as ps,    # [128,512]
            tc.tile_pool(name="psy", bufs=2, space="PSUM") as psy,  # [128,512]
            tc.tile_pool(name="pss", bufs=3, space="PSUM") as pss,  # [1,512]
        ):
            lcm = tc.For_i(0, loop_n, 1) if loop_n > 1 else contextlib.nullcontext()
            with lcm:
                # ---- phase-1-critical DMAs first: first K chunk + wk ----
                # split per c-tile so the first matmuls can start after ~1MB
                xkc0 = xs.tile([128, 16, 512], BF16, tag="xs", name="xkc0")
                wkb = []
                wkb0 = wkp.tile([128, 16, 128], BF16, tag="wk", name="wkb0")
                wkb.append(wkb0)
                for i4 in range(4):
                    sl = slice(4 * i4, 4 * i4 + 4)
                    nc.sync.dma_start(out=wkb0[:, sl, :], in_=wk_d[0][:, sl, :])
                    nc.sync.dma_start(out=xkc0[:, sl, :], in_=xk_d[0][:, sl, :])
                for j in range(1, 4):
                    wkt = wkp.tile([128, 16, 128], BF16, tag="wk", name=f"wkb{j}")
                    nc.sync.dma_start(out=wkt[:], in_=wk_d[j])
                    wkb.append(wkt)

                # ---- constants ----
                ones_colb = consts.tile([128, 1], BF16)
                nc.sync.dma_start(out=ones_colb[:], in_=onesb_d[:])
                ones_row = consts.tile([1, 512], F32R)
                nc.sync.dma_start(out=ones_row[:], in_=onesr_d[:])
                bq_sb = consts.tile([128, 16], F32)
                nc.sync.dma_start(out=bq_sb[:], in_=bq_d[:])
                bk_sb = consts.tile([128, 4], F32)
                nc.sync.dma_start(out=bk_sb[:], in_=bk_d[:])
                bv_sb = consts.tile([1, KVC], F32R)
                nc.sync.dma_start(out=bv_sb[:], in_=bv_d[:])
                bo_sb = consts.tile([1, C], F32R)
                nc.sync.dma_start(out=bo_sb[:], in_=bo_d[:])
                lnw_sb = consts.tile([128, 16], F32)
                nc.sync.dma_start(out=lnw_sb[:], in_=lnw_d[:])
                lnb_sb = consts.tile([128, 16], F32)
                nc.sync.dma_start(out=lnb_sb[:], in_=lnb_d[:])

                # ---- K projection: k^T layout [ch, keys], all 2048 keys ----
                ktf = []
                for g in range(4):
                    kt_t = ktf_pool.tile([128, T], BF16, tag="kt", name=f"ktf{g}")
                    ktf.append(kt_t)
                xq = None
                for ck in range(4):
                    if ck == 0:
                        xkc = xkc0
                    else:
                        xkc = xs.tile([128, 16, 512], BF16, tag="xs",
                                      name=f"xkc{ck}")
                        nc.sync.dma_start(out=xkc[:], in_=xk_d[ck])
                    if ck == 3:
                        # xq prefetch: big-pool slot is free, DMA is queued
                        # after the K chunks so it doesn't delay them
                        xq = big.tile([128, 16, R], BF16, tag="big", name="xq")
                        nc.sync.dma_start(out=xq[:], in_=xq_d[:])
                    for j in range(4):
                        ps_k = ps.tile([128, 512], F32, tag="ps", name=f"ps_k{ck}_{j}")
                        for i in range(16):
                            nc.tensor.matmul(
                                ps_k[:], wkb[j][:, i, :], xkc[:, i, :],
                                start=(i == 0), stop=(i == 15),
                                skip_group_check=True,
                            )
                        nc.scalar.activation(
                            ktf[j][:, ck * 512:(ck + 1) * 512], ps_k[:],
                            AF.Identity, bias=bk_sb[:, j:j + 1],
                        )

                # ---- V projection: natural layout [rows, ch], all 2048 rows ----
                wv_sb = big.tile([128, 16, KVC], BF16, tag="big", name="wv_sb")
                nc.sync.dma_start(out=wv_sb[:], in_=wv_d[:])
                # prefetch the first two Q-weight blocks so the Q projection
                # can start the moment V finishes
                wqb01 = []
                for j in range(2):
                    wqb = wqp.tile([128, 16, 128], BF16, tag="wq",
                                   name=f"wqb{j}")
                    nc.sync.dma_start(out=wqb[:], in_=wq_d[j])
                    wqb01.append(wqb)
                vf = []
                for ck in range(4):
                    xvc = xs.tile([128, 16, 512], BF16, tag="xs", name=f"xvc{ck}")
                    nc.sync.dma_start(out=xvc[:], in_=xv_d[ck])
                    for rl in range(4):
                        ps_v = ps.tile([128, 512], F32, tag="ps",
                                       name=f"ps_v{ck}_{rl}")
                        nc.tensor.matmul(
                            ps_v[:], ones_row[0:1, 0:128], bv_sb[0:1, :],
                            start=True, stop=False, skip_group_check=True,
                        )
                        for i in range(16):
                            nc.tensor.matmul(
                                ps_v[:], xvc[:, i, rl * 128:(rl + 1) * 128],
                                wv_sb[:, i, :], start=False, stop=(i == 15),
                                skip_group_check=True,
                            )
                        vt = vfb_pool.tile([128, KVC], BF16, tag="vf",
                                           name=f"vf{ck * 4 + rl}")
                        nc.vector.tensor_copy(out=vt[:], in_=ps_v[:])
                        vf.append(vt)

                # ---- Q projection (q^T layout, scale folded in by host) ----
                qt = []
                for j in range(16):
                    if j < 2:
                        wqb = wqb01[j]
                    else:
                        wqb = wqp.tile([128, 16, 128], BF16, tag="wq",
                                       name=f"wqb{j}")
                        nc.sync.dma_start(out=wqb[:], in_=wq_d[j])
                    ps_q = ps.tile([128, 512], F32, tag="ps", name=f"ps_q{j}")
                    for i in range(16):
                        nc.tensor.matmul(
                            ps_q[:], wqb[:, i, :], xq[:, i, :],
                            start=(i == 0), stop=(i == 15), skip_group_check=True,
                        )
                    qh = qtb_pool.tile([128, R], BF16, tag="qt", name=f"qt{j}")
                    nc.scalar.activation(
                        qh[:], ps_q[:], AF.Identity, bias=bq_sb[:, j:j + 1]
                    )
                    qt.append(qh)

                # ---- prefetch first Wo blocks (pool slots free during attn) ----
                wob = [None] * 4
                for jb in range(2):
                    wob[jb] = big.tile([128, 16, 512], BF16, tag="big",
                                       name=f"wob{jb}")
                    nc.sync.dma_start(out=wob[jb][:], in_=wo_d[jb])

                # ---- attention (LayerNorm sums folded into head loop) ----
                # Head h-1's softmax-sum matmul and normalization are emitted
                # in the middle of head h's score/AV stream, so the PE never
                # stalls on the DVE tree tail or the reciprocal.
                yt = ytp.tile([128, 16, R], BF16)
                ps_mu = pss.tile([1, 512], F32, tag="pss")
                ps_sq = pss.tile([1, 512], F32, tag="pss")
                rS_h = [None] * H
                ps_y_h = [None] * H
                ssum_h = [None] * H

                def s_and_recip(hp):
                    ps_S = pss.tile([1, 512], F32, tag="pss", name=f"ps_S{hp}")
                    nc.tensor.matmul(
                        ps_S[:], ones_colb[:], ssum_h[hp][:],
                        start=True, stop=True, skip_group_check=True,
                    )
                    rS = s1.tile([1, 512], F32R, tag="s1", name=f"rS{hp}")
                    with nc.allow_low_precision("fp32r rounding for bcast matmul"):
                        nc.vector.reciprocal(rS[:], ps_S[:])
                    rS_h[hp] = rS

                sum_mu = None
                sum_sq = None
                ps_mu15 = None
                ps_sq15 = None

                def apply_norm(hp):
                    nonlocal sum_mu, sum_sq, ps_mu15, ps_sq15
                    ps_r = ps.tile([128, 512], F32, tag="ps", name=f"ps_r{hp}")
                    nc.tensor.matmul(
                        ps_r[:], ones_row[0:1, 0:128], rS_h[hp][:],
                        start=True, stop=True, skip_group_check=True,
                    )
                    rSb = blkf.tile([128, 512], F32, tag="blkf", name=f"rSb{hp}")
                    nc.vector.tensor_copy(out=rSb[:], in_=ps_r[:])
                    nc.vector.tensor_tensor(
                        yt[:, hp, :], ps_y_h[hp][:], rSb[:], op=ALU.mult
                    )
                    # LayerNorm running sums for this head's channels.
                    # Heads 0-14 accumulate in ps_mu/ps_sq (closed at 14 so the
                    # sums are staged to SBUF during head 15); head 15 gets its
                    # own single-matmul stats so the final chain is short.
                    ysq = blkf.tile([128, 512], BF16, tag="blkf", name=f"ysq{hp}")
                    nc.gpsimd.tensor_tensor(
                        ysq[:], yt[:, hp, :], yt[:, hp, :], op=ALU.mult
                    )
                    if hp <= 14:
                        nc.tensor.matmul(
                            ps_mu[:], ones_colb[:], yt[:, hp, :],
                            start=(hp == 0), stop=(hp == 14),
                            skip_group_check=True,
                        )
                        nc.tensor.matmul(
                            ps_sq[:], ones_colb[:], ysq[:],
                            start=(hp == 0), stop=(hp == 14),
                            skip_group_check=True,
                        )
                        if hp == 14:
                            sum_mu = s1.tile([1, 512], F32, tag="s1",
                                             name="sum_mu")
                            nc.vector.tensor_copy(out=sum_mu[:], in_=ps_mu[:])
                            sum_sq = s1.tile([1, 512], F32, tag="s1",
                                             name="sum_sq")
                            nc.vector.tensor_copy(out=sum_sq[:], in_=ps_sq[:])
                    else:
                        ps_mu15 = psy.tile([1, 512], F32, tag="psy",
                                           name="ps_mu15")
                        nc.tensor.matmul(
                            ps_mu15[:], ones_colb[:], yt[:, hp, :],
                            start=True, stop=True, skip_group_check=True,
                        )
                        ps_sq15 = psy.tile([1, 512], F32, tag="psy",
                                           name="ps_sq15")
                        nc.tensor.matmul(
                            ps_sq15[:], ones_colb[:], ysq[:],
                            start=True, stop=True, skip_group_check=True,
                        )

                for h in range(H):
                    g = h // 4
                    ps_y = psy.tile([128, 512], F32, tag="psy", name=f"ps_y{h}")
                    ps_y_h[h] = ps_y
                    att = []
                    for kt in range(16):
                        ps_s = ps.tile([128, 512], F32, tag="ps",
                                       name=f"ps_s{h}_{kt}")
                        nc.tensor.matmul(
                            ps_s[:], ktf[g][:, kt * 128:(kt + 1) * 128], qt[h][:],
                            start=True, stop=True, skip_group_check=True,
                        )
                        a = att_pool.tile([128, 512], BF16, tag="att",
                                          name=f"att{h}_{kt}")
                        nc.scalar.activation(a[:], ps_s[:], AF.Exp)
                        att.append(a)
                        nc.tensor.matmul(
                            ps_y[:], vf[kt][:, g * 128:(g + 1) * 128], a[:],
                            start=(kt == 0), stop=(kt == 15),
                            skip_group_check=True,
                        )
                        # softmax-sum tree, in place on att tiles (DVE)
                        if kt % 2 == 1:
                            nc.vector.tensor_tensor(
                                att[kt - 1][:], att[kt - 1][:], att[kt][:],
                                op=ALU.add,
                            )
                        if h > 0 and kt == 7:
                            s_and_recip(h - 1)
                        if h > 0 and kt == 11:
                            apply_norm(h - 1)
                    for i in range(4):
                        nc.vector.tensor_tensor(
                            att[4 * i][:], att[4 * i][:], att[4 * i + 2][:],
                            op=ALU.add,
                        )
                    nc.vector.tensor_tensor(
                        att[0][:], att[0][:], att[4][:], op=ALU.add
                    )
                    nc.vector.tensor_tensor(
                        att[8][:], att[8][:], att[12][:], op=ALU.add
                    )
                    ssum = att_pool.tile([128, 512], BF16, tag="ssum",
                                         name=f"ssum{h}", bufs=2)
                    nc.vector.tensor_tensor(
                        ssum[:], att[0][:], att[8][:], op=ALU.add
                    )
                    ssum_h[h] = ssum
                s_and_recip(H - 1)
                apply_norm(H - 1)

                # ---- LayerNorm stats + apply ----
                mu = s1.tile([1, 512], F32R, tag="s1")
                nc.vector.tensor_tensor(mu[:], sum_mu[:], ps_mu15[:], op=ALU.add)
                with nc.allow_low_precision("fp32r stats"):
                    nc.vector.tensor_scalar_mul(mu[:], mu[:], 1.0 / C)
                m2 = s1.tile([1, 512], F32, tag="s1")
                nc.vector.tensor_tensor(m2[:], sum_sq[:], ps_sq15[:], op=ALU.add)
                nc.vector.tensor_scalar_mul(m2[:], m2[:], 1.0 / C)
                var = s1.tile([1, 512], F32, tag="s1")
                nc.vector.tensor_tensor(var[:], mu[:], mu[:], op=ALU.mult)
                nc.vector.tensor_tensor(var[:], m2[:], var[:], op=ALU.subtract)
                nc.vector.tensor_scalar_add(var[:], var[:], EPS)
                sd = s1.tile([1, 512], F32, tag="s1")
                nc.scalar.activation(sd[:], var[:], AF.Sqrt)
                rstd = s1.tile([1, 512], F32R, tag="s1")
                with nc.allow_low_precision("fp32r rounding for bcast matmul"):
                    nc.vector.reciprocal(rstd[:], sd[:])
                # broadcast mu and rstd across partitions (bf16 for LN apply)
                ps_r = ps.tile([128, 512], F32, tag="ps", name="ps_rmu")
                nc.tensor.matmul(
                    ps_r[:], ones_row[0:1, 0:128], mu[:], start=True, stop=True,
                    skip_group_check=True,
                )
                mub = blkf.tile([128, 512], BF16, tag="blkf", name="mub")
                nc.vector.tensor_copy(out=mub[:], in_=ps_r[:])
                ps_r2 = ps.tile([128, 512], F32, tag="ps", name="ps_rsd")
                nc.tensor.matmul(
                    ps_r2[:], ones_row[0:1, 0:128], rstd[:], start=True, stop=True,
                    skip_group_check=True,
                )
                rstdb = blkf.tile([128, 512], BF16, tag="blkf", name="rstdb")
                nc.vector.tensor_copy(out=rstdb[:], in_=ps_r2[:])
                # ---- LN apply interleaved with out-proj block jb=0 ----
                # jb=0's four accumulation groups consume each normalized
                # ct-slice as soon as it is written, hiding the DVE LN chain
                # under PE matmuls.
                ps_o0 = []
                for m in range(4):
                    pool0 = ps if m < 3 else psy
                    ps_o = pool0.tile([128, 512], F32,
                                      tag=("ps" if m < 3 else "psy"),
                                      name=f"ps_o0_{m}")
                    nc.tensor.matmul(
                        ps_o[:], ones_row[0:1, 0:128], bo_sb[0:1, 0:512],
                        start=True, stop=False, skip_group_check=True,
                    )
                    ps_o0.append(ps_o)
                for ct in range(16):
                    scr = blkf.tile([128, 512], BF16, tag="blkf", name=f"scr{ct}")
                    nc.gpsimd.tensor_tensor(
                        scr[:], yt[:, ct, :], mub[:], op=ALU.subtract
                    )
                    nc.vector.tensor_tensor(scr[:], scr[:], rstdb[:], op=ALU.mult)
                    nc.vector.tensor_scalar(
                        yt[:, ct, :], scr[:],
                        lnw_sb[:, ct:ct + 1], lnb_sb[:, ct:ct + 1],
                        op0=ALU.mult, op1=ALU.add,
                    )
                    for m in range(4):
                        nc.tensor.matmul(
                            ps_o0[m][:], yt[:, ct, m * 128:(m + 1) * 128],
                            wob[0][:, ct, :], start=False, stop=(ct == 15),
                            skip_group_check=True,
                        )
                for m in range(4):
                    osb = blkf.tile([128, 512], F32, tag="osb",
                                    name=f"osb0_{m}", bufs=3)
                    if m % 2 == 0:
                        nc.scalar.activation(osb[:], ps_o0[m][:], AF.Copy)
                    else:
                        nc.vector.tensor_copy(out=osb[:], in_=ps_o0[m][:])
                    nc.sync.dma_start(
                        out=out_d[m * 128:(m + 1) * 128, 0:512],
                        in_=osb[:],
                    )

                # ---- remaining output projection blocks ----
                for jb in range(1, 4):
                    if wob[jb] is None:
                        wob[jb] = big.tile([128, 16, 512], BF16, tag="big",
                                           name=f"wob{jb}")
                        nc.sync.dma_start(out=wob[jb][:], in_=wo_d[jb])
                    for m in range(4):
                        ps_o = ps.tile([128, 512], F32, tag="ps",
                                       name=f"ps_o{jb}_{m}")
                        nc.tensor.matmul(
                            ps_o[:], ones_row[0:1, 0:128],
                            bo_sb[0:1, jb * 512:(jb + 1) * 512],
                            start=True, stop=False, skip_group_check=True,
                        )
                        for i in range(16):
                            nc.tensor.matmul(
                                ps_o[:], yt[:, i, m * 128:(m + 1) * 128],
                                wob[jb][:, i, :], start=False, stop=(i == 15),
                                skip_group_check=True,
                            )
                        osb = blkf.tile([128, 512], F32, tag="osb",
                                        name=f"osb{jb}_{m}", bufs=3)
                        if jb == 3:
                            nc.scalar.activation(
                                osb[:, 0:256], ps_o[:, 0:256], AF.Copy
                            )
                            nc.sync.dma_start(
                                out=out_d[m * 128:(m + 1) * 128,
                                          jb * 512:jb * 512 + 256],
                                in_=osb[:, 0:256],
                            )
                            nc.vector.tensor_copy(
                                out=osb[:, 256:512], in_=ps_o[:, 256:512]
                            )
                            nc.sync.dma_start(
                                out=out_d[m * 128:(m + 1) * 128,
                                          jb * 512 + 256:(jb + 1) * 512],
                                in_=osb[:, 256:512],
                            )
                        else:
                            if m % 2 == 0:
                                nc.scalar.activation(osb[:], ps_o[:], AF.Copy)
                            else:
                                nc.vector.tensor_copy(out=osb[:], in_=ps_o[:])
                            nc.sync.dma_start(
                                out=out_d[m * 128:(m + 1) * 128,
                                          jb * 512:(jb + 1) * 512],
                                in_=osb[:],
                            )

    nc.compile()
    return nc


_NC_CACHE = None
LAST_RES = None


def _get_nc():
    global _NC_CACHE
    if _NC_CACHE is None:
        _NC_CACHE = build_kernel()
    return _NC_CACHE


def _prep_shared(Wq, bq, Wk, bk, Wv, bv, ln_w, ln_b, Wo, bo):
    import ml_dtypes

    BF = ml_dtypes.bfloat16
    s = np.float32(SCALE)
    WqT = np.ascontiguousarray(Wq.T) * s  # [c, ch], scale folded into q
    # wq[j, p, i, cc] = WqT[i*128+p, j*128+cc]
    wq = np.ascontiguousarray(
        WqT.reshape(16, 128, 16, 128).transpose(2, 1, 0, 3).astype(BF)
    )
    WkT = np.ascontiguousarray(Wk.T)  # [2048, 512]
    wk = np.ascontiguousarray(
        WkT.reshape(16, 128, 4, 128).transpose(2, 1, 0, 3).astype(BF)
    )
    WvT = np.ascontiguousarray(Wv.T)  # [2048, 512]
    wv = np.ascontiguousarray(
        WvT.reshape(16, 128, KVC).transpose(1, 0, 2).astype(BF)
    )
    WoT = np.ascontiguousarray(Wo.T)  # [2048, 2048]
    wo = np.ascontiguousarray(
        WoT.reshape(16, 128, 4, 512).transpose(2, 1, 0, 3).astype(BF)
    )
    return {
        "wq": wq,
        "wk": wk,
        "wv": wv,
        "wo": wo,
        "bq": np.ascontiguousarray((bq * s).reshape(16, 128).T),
        "bk": np.ascontiguousarray(bk.reshape(4, 128).T),
        "bv": np.ascontiguousarray(bv.reshape(1, KVC)),
        "bo": np.ascontiguousarray(bo.reshape(1, C)),
        "lnw": np.ascontiguousarray(ln_w.reshape(16, 128).T),
        "lnb": np.ascontiguousarray(ln_b.reshape(16, 128).T),
        "onesb": np.ones((128, 1), BF),
        "onesr": np.ones((1, 512), np.float32),
    }


def _xt_full_tiled(x):
    # x [T, C] -> x^T tiled chunk-major [4, 128, 16, 512] bf16
    import ml_dtypes

    xT = np.ascontiguousarray(x.T)  # [C, T]
    return np.ascontiguousarray(
        xT.reshape(16, 128, 4, 512).transpose(2, 1, 0, 3).astype(ml_dtypes.bfloat16)
    )


def _xt_tiled(x):
    # x [R, C] -> x^T tiled [128, 16, R] bf16
    import ml_dtypes

    xT = np.ascontiguousarray(x.T)  # [C, R]
    return np.ascontiguousarray(
        xT.reshape(16, 128, R).transpose(1, 0, 2).astype(ml_dtypes.bfloat16)
    )


def kernel(
    query, key, value, Wq, bq, Wk, bk, Wv, bv, ln_w, ln_b, Wo, bo
):
    query = np.asarray(query, np.float32)
    key = np.asarray(key, np.float32)
    value = np.asarray(value, np.float32)

    nc = _get_nc()
    shared = _prep_shared(
        np.asarray(Wq, np.float32), np.asarray(bq, np.float32),
        np.asarray(Wk, np.float32), np.asarray(bk, np.float32),
        np.asarray(Wv, np.float32), np.asarray(bv, np.float32),
        np.asarray(ln_w, np.float32), np.asarray(ln_b, np.float32),
        np.asarray(Wo, np.float32), np.asarray(bo, np.float32),
    )

    xk_b = [_xt_full_tiled(key[b]) for b in range(B)]
    xv_b = [_xt_full_tiled(value[b]) for b in range(B)]

    in_maps = []
    for c in range(N_CORES):
        b = c // 4
        r0 = (c % 4) * R
        m = dict(shared)
        m["xq"] = _xt_tiled(query[b, r0:r0 + R, :])
        m["xk"] = xk_b[b]
        m["xv"] = xv_b[b]
        in_maps.append(m)

    res = run_bass_kernel_spmd(nc, in_maps, core_ids=list(range(N_CORES)))
    global LAST_RES
    LAST_RES = res

    out = np.empty((B, T, C), np.float32)
    for c in range(N_CORES):
        b = c // 4
        r0 = (c % 4) * R
        out[b, r0:r0 + R, :] = res.results[c]["out"]
    return out


# revision 38
# speedup vs baseline: 1.0082x; 1.0082x over previous
"""BitMGQA forward on 8 trn2 NeuronCores, collective-free.

Core c owns batch b=c//4 and query rows (c%4)*512:(c%4+1)*512. Every core
recomputes the K/V projections for ALL 2048 keys of its batch (replicated
across the 4 cores of a batch group), so there is no cross-core
communication at all: no collectives, no rendezvous — each core's NEFF
runs to completion independently of the other cores' launch times.
Outputs are disjoint row slices -> host concat.

All matmuls are bf16 operands with f32 PSUM accumulation; LayerNorm
statistics and softmax normalization are computed in f32.
"""

import contextlib

import numpy as np

import concourse.bacc as bacc
import concourse.mybir as mybir
import concourse.tile as tile
from concourse.bass_utils import run_bass_kernel_spmd

B, T, C = 2, 2048, 2048
H, KV = 16, 4
HD = C // H  # 128
KVC = HD * KV  # 512
EPS = 1e-5
R = 512  # query rows per core
N_CORES = 8
SCALE = 1.0 / np.sqrt(HD)

F32 = mybir.dt.float32
F32R = mybir.dt.float32r
BF16 = mybir.dt.bfloat16
AF = mybir.ActivationFunctionType
ALU = mybir.AluOpType


def build_kernel(loop_n=1):
    nc = bacc.Bacc(
        "TRN2", target_bir_lowering=False, debug=False, num_devices=N_CORES
    )

    # Per-core inputs (host pre-transposed/tiled, see kernel() below)
    xq_d = nc.dram_tensor("xq", [128, 16, R], BF16, kind="ExternalInput").ap()
    # key/value of this core's batch, ^T tiled, chunk-major (chunk=512 keys)
    xk_d = nc.dram_tensor("xk", [4, 128, 16, 512], BF16, kind="ExternalInput").ap()
    xv_d = nc.dram_tensor("xv", [4, 128, 16, 512], BF16, kind="ExternalInput").ap()
    wq_d = nc.dram_tensor("wq", [16, 128, 16, 128], BF16, kind="ExternalInput").ap()
    wk_d = nc.dram_tensor("wk", [4, 128, 16, 128], BF16, kind="ExternalInput").ap()
    wv_d = nc.dram_tensor("wv", [128, 16, KVC], BF16, kind="ExternalInput").ap()
    wo_d = nc.dram_tensor("wo", [4, 128, 16, 512], BF16, kind="ExternalInput").ap()
    bq_d = nc.dram_tensor("bq", [128, 16], F32, kind="ExternalInput").ap()
    bk_d = nc.dram_tensor("bk", [128, 4], F32, kind="ExternalInput").ap()
    bv_d = nc.dram_tensor("bv", [1, KVC], F32R, kind="ExternalInput").ap()
    bo_d = nc.dram_tensor("bo", [1, C], F32R, kind="ExternalInput").ap()
    lnw_d = nc.dram_tensor("lnw", [128, 16], F32, kind="ExternalInput").ap()
    lnb_d = nc.dram_tensor("lnb", [128, 16], F32, kind="ExternalInput").ap()
    onesb_d = nc.dram_tensor("onesb", [128, 1], BF16, kind="ExternalInput").ap()
    onesr_d = nc.dram_tensor("onesr", [1, 512], F32R, kind="ExternalInput").ap()

    out_d = nc.dram_tensor("out", [R, C], F32, kind="ExternalOutput").ap()

    with tile.TileContext(nc) as tc:
        with (
            tc.tile_pool(name="consts", bufs=1) as consts,
            tc.tile_pool(name="wkp", bufs=4) as wkp,       # wk blocks [128,16,128]
            tc.tile_pool(name="big", bufs=2) as big,       # wv + wo blocks [128,16,512]
            tc.tile_pool(name="xs", bufs=2) as xs,         # x chunks [128,16,512]
            tc.tile_pool(name="wqp", bufs=2) as wqp,       # wq blocks [128,16,128]
            tc.tile_pool(name="qtb", bufs=16) as qtb_pool, # q^T per head [128,512]
            tc.tile_pool(name="ktf", bufs=4) as ktf_pool,  # k^T per group [128,2048]
            tc.tile_pool(name="vfb", bufs=16) as vfb_pool, # v tiles [128,512]
            tc.tile_pool(name="ytp", bufs=1) as ytp,       # y^T [128,16,512] bf16
            tc.tile_pool(name="att", bufs=17) as att_pool, # att tiles [128,512] bf16
            tc.tile_pool(name="blkf", bufs=4) as blkf,     # f32 scratch [128,512]
            tc.tile_pool(name="s1", bufs=4) as s1,         # [1,512] f32
            tc.tile_pool(name="ps", bufs=4, space="PSUM") as ps,    # [128,512]
            tc.tile_pool(name="psy", bufs=2, space="PSUM") as psy,  # [128,512]
            tc.tile_pool(name="pss", bufs=2, space="PSUM") as pss,  # [1,512]
        ):
            lcm = tc.For_i(0, loop_n, 1) if loop_n > 1 else contextlib.nullcontext()
            with lcm:
                # ---- phase-1-critical DMAs first: first K chunk + wk ----
                # split per c-tile so the first matmuls can start after ~1MB
                xkc0 = xs.tile([128, 16, 512], BF16, tag="xs", name="xkc0")
                wkb = []
                wkb0 = wkp.tile([128, 16, 128], BF16, tag="wk", name="wkb0")
                wkb.append(wkb0)
                for i4 in range(4):
                    sl = slice(4 * i4, 4 * i4 + 4)
                    nc.sync.dma_start(out=wkb0[:, sl, :], in_=wk_d[0][:, sl, :])
                    nc.sync.dma_start(out=xkc0[:, sl, :], in_=xk_d[0][:, sl, :])
                for j in range(1, 4):
                    wkt = wkp.tile([128, 16, 128], BF16, tag="wk", name=f"wkb{j}")
                    nc.sync.dma_start(out=wkt[:], in_=wk_d[j])
                    wkb.append(wkt)

                # ---- constants ----
                ones_colb = consts.tile([128, 1], BF16)
                nc.sync.dma_start(out=ones_colb[:], in_=onesb_d[:])
                ones_row = consts.tile([1, 512], F32R)
                nc.sync.dma_start(out=ones_row[:], in_=onesr_d[:])
                bq_sb = consts.tile([128, 16], F32)
                nc.sync.dma_start(out=bq_sb[:], in_=bq_d[:])
                bk_sb = consts.tile([128, 4], F32)
                nc.sync.dma_start(out=bk_sb[:], in_=bk_d[:])
                bv_sb = consts.tile([1, KVC], F32R)
                nc.sync.dma_start(out=bv_sb[:], in_=bv_d[:])
                bo_sb = consts.tile([1, C], F32R)
                nc.sync.dma_start(out=bo_sb[:], in_=bo_d[:])
                lnw_sb = consts.tile([128, 16], F32)
                nc.sync.dma_start(out=lnw_sb[:], in_=lnw_d[:])
                lnb_sb = consts.tile([128, 16], F32)
                nc.sync.dma_start(out=lnb_sb[:], in_=lnb_d[:])

                # ---- K projection: k^T layout [ch, keys], all 2048 keys ----
                ktf = []
                for g in range(4):
                    kt_t = ktf_pool.tile([128, T], BF16, tag="kt", name=f"ktf{g}")
                    ktf.append(kt_t)
                xq = None
                for ck in range(4):
                    if ck == 0:
                        xkc = xkc0
                    else:
                        xkc = xs.tile([128, 16, 512], BF16, tag="xs",
                                      name=f"xkc{ck}")
                        nc.sync.dma_start(out=xkc[:], in_=xk_d[ck])
                    if ck == 3:
                        # xq prefetch: big-pool slot is free, DMA is queued
                        # after the K chunks so it doesn't delay them
                        xq = big.tile([128, 16, R], BF16, tag="big", name="xq")
                        nc.sync.dma_start(out=xq[:], in_=xq_d[:])
                    for j in range(4):
                        ps_k = ps.tile([128, 512], F32, tag="ps", name=f"ps_k{ck}_{j}")
                        for i in range(16):
                            nc.tensor.matmul(
                                ps_k[:], wkb[j][:, i, :], xkc[:, i, :],
                                start=(i == 0), stop=(i == 15),
                                skip_group_check=True,
                            )
                        nc.scalar.activation(
                            ktf[j][:, ck * 512:(ck + 1) * 512], ps_k[:],
                            AF.Identity, bias=bk_sb[:, j:j + 1],
                        )

                # ---- V projection: natural layout [rows, ch], all 2048 rows ----
                wv_sb = big.tile([128, 16, KVC], BF16, tag="big", name="wv_sb")
                nc.sync.dma_start(out=wv_sb[:], in_=wv_d[:])
                # prefetch the first two Q-weight blocks so the Q projection
                # can start the moment V finishes
                wqb01 = []
                for j in range(2):
                    wqb = wqp.tile([128, 16, 128], BF16, tag="wq",
                                   name=f"wqb{j}")
                    nc.sync.dma_start(out=wqb[:], in_=wq_d[j])
                    wqb01.append(wqb)
                vf = []
                for ck in range(4):
                    xvc = xs.tile([128, 16, 512], BF16, tag="xs", name=f"xvc{ck}")
                    nc.sync.dma_start(out=xvc[:], in_=xv_d[ck])
                    for rl in range(4):
                        ps_v = ps.tile([128, 512], F32, tag="ps",
                                       name=f"ps_v{ck}_{rl}")
                        nc.tensor.matmul(
                            ps_v[:], ones_row[0:1, 0:128], bv_sb[0:1, :],
                            start=True, stop=False, skip_group_check=True,
                        )
                        for i in range(16):
                            nc.tensor.matmul(
                                ps_v[:], xvc[:, i, rl * 128:(rl + 1) * 128],
                                wv_sb[:, i, :], start=False, stop=(i == 15),
                                skip_group_check=True,
                            )
                        vt = vfb_pool.tile([128, KVC], BF16, tag="vf",
                                           name=f"vf{ck * 4 + rl}")
                        nc.vector.tensor_copy(out=vt[:], in_=ps_v[:])
                        vf.append(vt)

                # ---- Q projection (q^T layout, scale folded in by host) ----
                qt = []
                for j in range(16):
                    if j < 2:
                        wqb = wqb01[j]
                    else:
                        wqb = wqp.tile([128, 16, 128], BF16, tag="wq",
                                       name=f"wqb{j}")
                        nc.sync.dma_start(out=wqb[:], in_=wq_d[j])
                    ps_q = ps.tile([128, 512], F32, tag="ps", name=f"ps_q{j}")
                    for i in range(16):
                        nc.tensor.matmul(
                            ps_q[:], wqb[:, i, :], xq[:, i, :],
                            start=(i == 0), stop=(i == 15), skip_group_check=True,
                        )
                    qh = qtb_pool.tile([128, R], BF16, tag="qt", name=f"qt{j}")
                    nc.scalar.activation(
                        qh[:], ps_q[:], AF.Identity, bias=bq_sb[:, j:j + 1]
                    )
                    qt.append(qh)

                # ---- prefetch first Wo blocks (pool slots free during attn) ----
                wob = [None] * 4
                for jb in range(2):
                    wob[jb] = big.tile([128, 16, 512], BF16, tag="big",
                                       name=f"wob{jb}")
                    nc.sync.dma_start(out=wob[jb][:], in_=wo_d[jb])

                # ---- attention (LayerNorm sums folded into head loop) ----
                # Head h-1's softmax-sum matmul and normalization are emitted
                # in the middle of head h's score/AV stream, so the PE never
                # stalls on the DVE tree tail or the reciprocal.
                yt = ytp.tile([128, 16, R], BF16)
                # mu and sq accumulators packed into ONE psum bank at
                # partition offsets 0 and 32 (sq matmuls use tile_position)
                ps_musq = pss.tile([128, 512], F32, tag="pss", name="ps_musq")
                ps_mu = ps_musq[0:1, :]
                ps_sq = ps_musq[32:33, :]
                rS_h = [None] * H
                ps_y_h = [None] * H
                ssum_h = [None] * H

                def s_and_recip(hp):
                    ps_S = pss.tile([1, 512], F32, tag="pss", name=f"ps_S{hp}")
                    nc.tensor.matmul(
                        ps_S[:], ones_colb[:], ssum_h[hp][:],
                        start=True, stop=True, skip_group_check=True,
                    )
                    rS = s1.tile([1, 512], F32R, tag="s1", name=f"rS{hp}")
                    with nc.allow_low_precision("fp32r rounding for bcast matmul"):
                        nc.vector.reciprocal(rS[:], ps_S[:])
                    rS_h[hp] = rS

                sum_mu = None
                sum_sq = None
                ps_mu15 = None
                ps_sq15 = None

                def apply_norm(hp):
                    nonlocal sum_mu, sum_sq, ps_mu15, ps_sq15
                    ps_r = ps.tile([128, 512], F32, tag="ps", name=f"ps_r{hp}")
                    nc.tensor.matmul(
                        ps_r[:], ones_row[0:1, 0:128], rS_h[hp][:],
                        start=True, stop=True, skip_group_check=True,
                    )
                    rSb = blkf.tile([128, 512], F32, tag="blkf", name=f"rSb{hp}")
                    nc.vector.tensor_copy(out=rSb[:], in_=ps_r[:])
                    nc.vector.tensor_tensor(
                        yt[:, hp, :], ps_y_h[hp][:], rSb[:], op=ALU.mult
                    )
                    # LayerNorm running sums for this head's channels.
                    # Heads 0-14 accumulate in ps_mu/ps_sq (closed at 14 so the
                    # sums are staged to SBUF during head 15); head 15 gets its
                    # own single-matmul stats so the final chain is short.
                    ysq = blkf.tile([128, 512], BF16, tag="blkf", name=f"ysq{hp}")
                    nc.gpsimd.tensor_tensor(
                        ysq[:], yt[:, hp, :], yt[:, hp, :], op=ALU.mult
                    )
                    if hp <= 14:
                        nc.tensor.matmul(
                            ps_mu[:], ones_colb[:], yt[:, hp, :],
                            start=(hp == 0), stop=(hp == 14),
                            skip_group_check=True,
                        )
                        nc.tensor.matmul(
                            ps_sq[:], ones_colb[:], ysq[:],
                            start=(hp == 0), stop=(hp == 14),
                            skip_group_check=True, tile_position=(0, 32),
                        )
                        if hp == 14:
                            sum_mu = s1.tile([1, 512], F32, tag="s1",
                                             name="sum_mu")
                            nc.vector.tensor_copy(out=sum_mu[:], in_=ps_mu[:])
                            sum_sq = s1.tile([1, 512], F32, tag="s1",
                                             name="sum_sq")
                            nc.vector.tensor_copy(out=sum_sq[:], in_=ps_sq[:])
                    else:
                        ps_mu15 = psy.tile([1, 512], F32, tag="psy",
                                           name="ps_mu15")
                        nc.tensor.matmul(
                            ps_mu15[:], ones_colb[:], yt[:, hp, :],
                            start=True, stop=True, skip_group_check=True,
                        )
                        ps_sq15 = psy.tile([1, 512], F32, tag="psy",
                                           name="ps_sq15")
                        nc.tensor.matmul(
                            ps_sq15[:], ones_colb[:], ysq[:],
                            start=True, stop=True, skip_group_check=True,
                        )

                for h in range(H):
                    g = h // 4
                    ps_y = psy.tile([128, 512], F32, tag="psy", name=f"ps_y{h}")
                    ps_y_h[h] = ps_y
                    att = []
                    for kt in range(16):
                        ps_s = ps.tile([128, 512], F32, tag="ps",
                                       name=f"ps_s{h}_{kt}")
                        nc.tensor.matmul(
                            ps_s[:], ktf[g][:, kt * 128:(kt + 1) * 128], qt[h][:],
                            start=True, stop=True, skip_group_check=True,
                        )
                        a = att_pool.tile([128, 512], BF16, tag="att",
                                          name=f"att{h}_{kt}")
                        nc.scalar.activation(a[:], ps_s[:], AF.Exp)
                        att.append(a)
                        nc.tensor.matmul(
                            ps_y[:], vf[kt][:, g * 128:(g + 1) * 128], a[:],
                            start=(kt == 0), stop=(kt == 15),
                            skip_group_check=True,
                        )
                        # softmax-sum tree, in place on att tiles (DVE)
                        if kt % 2 == 1:
                            nc.vector.tensor_tensor(
                                att[kt - 1][:], att[kt - 1][:], att[kt][:],
                                op=ALU.add,
                            )
                        if h > 0 and kt == 7:
                            s_and_recip(h - 1)
                        if h > 0 and kt == 11:
                            apply_norm(h - 1)
                    for i in range(4):
                        nc.vector.tensor_tensor(
                            att[4 * i][:], att[4 * i][:], att[4 * i + 2][:],
                            op=ALU.add,
                        )
                    nc.vector.tensor_tensor(
                        att[0][:], att[0][:], att[4][:], op=ALU.add
                    )
                    nc.vector.tensor_tensor(
                        att[8][:], att[8][:], att[12][:], op=ALU.add
                    )
                    ssum = att_pool.tile([128, 512], BF16, tag="ssum",
                                         name=f"ssum{h}", bufs=2)
                    nc.vector.tensor_tensor(
                        ssum[:], att[0][:], att[8][:], op=ALU.add
                    )
                    ssum_h[h] = ssum
                s_and_recip(H - 1)
                apply_norm(H - 1)

                # ---- LayerNorm stats + apply ----
                mu = s1.tile([1, 512], F32R, tag="s1")
                nc.vector.tensor_tensor(mu[:], sum_mu[:], ps_mu15[:], op=ALU.add)
                with nc.allow_low_precision("fp32r stats"):
                    nc.vector.tensor_scalar_mul(mu[:], mu[:], 1.0 / C)
                m2 = s1.tile([1, 512], F32, tag="s1")
                nc.vector.tensor_tensor(m2[:], sum_sq[:], ps_sq15[:], op=ALU.add)
                nc.vector.tensor_scalar_mul(m2[:], m2[:], 1.0 / C)
                var = s1.tile([1, 512], F32, tag="s1")
                nc.vector.tensor_tensor(var[:], mu[:], mu[:], op=ALU.mult)
                nc.vector.tensor_tensor(var[:], m2[:], var[:], op=ALU.subtract)
                nc.vector.tensor_scalar_add(var[:], var[:], EPS)
                sd = s1.tile([1, 512], F32, tag="s1")
                nc.scalar.activation(sd[:], var[:], AF.Sqrt)
                rstd = s1.tile([1, 512], F32R, tag="s1")
                with nc.allow_low_precision("fp32r rounding for bcast matmul"):
                    nc.vector.reciprocal(rstd[:], sd[:])
                # broadcast mu and rstd across partitions (bf16 for LN apply)
                ps_r = ps.tile([128, 512], F32, tag="ps", name="ps_rmu")
                nc.tensor.matmul(
                    ps_r[:], ones_row[0:1, 0:128], mu[:], start=True, stop=True,
                    skip_group_check=True,
                )
                mub = blkf.tile([128, 512], BF16, tag="blkf", name="mub")
                nc.vector.tensor_copy(out=mub[:], in_=ps_r[:])
                ps_r2 = ps.tile([128, 512], F32, tag="ps", name="ps_rsd")
                nc.tensor.matmul(
                    ps_r2[:], ones_row[0:1, 0:128], rstd[:], start=True, stop=True,
                    skip_group_check=True,
                )
                rstdb = blkf.tile([128, 512], BF16, tag="blkf", name="rstdb")
                nc.vector.tensor_copy(out=rstdb[:], in_=ps_r2[:])
                # ---- LN apply interleaved with out-proj block jb=0 ----
                # jb=0's four accumulation groups consume each normalized
                # ct-slice as soon as it is written, hiding the DVE LN chain
                # under PE matmuls.
                ps_o0 = []
                for m in range(4):
                    pool0 = ps if m < 3 else psy
                    ps_o = pool0.tile([128, 512], F32,
                                      tag=("ps" if m < 3 else "psy"),
                                      name=f"ps_o0_{m}")
                    nc.tensor.matmul(
                        ps_o[:], ones_row[0:1, 0:128], bo_sb[0:1, 0:512],
                        start=True, stop=False, skip_group_check=True,
                    )
                    ps_o0.append(ps_o)
                for ct in range(16):
                    scr = blkf.tile([128, 512], BF16, tag="blkf", name=f"scr{ct}")
                    nc.gpsimd.tensor_tensor(
                        scr[:], yt[:, ct, :], mub[:], op=ALU.subtract
                    )
                    nc.vector.tensor_tensor(scr[:], scr[:], rstdb[:], op=ALU.mult)
                    nc.vector.tensor_scalar(
                        yt[:, ct, :], scr[:],
                        lnw_sb[:, ct:ct + 1], lnb_sb[:, ct:ct + 1],
                        op0=ALU.mult, op1=ALU.add,
                    )
                    for m in range(4):
                        nc.tensor.matmul(
                            ps_o0[m][:], yt[:, ct, m * 128:(m + 1) * 128],
                            wob[0][:, ct, :], start=False, stop=(ct == 15),
                            skip_group_check=True,
                        )
                for m in range(4):
                    osb = blkf.tile([128, 512], F32, tag="osb",
                                    name=f"osb0_{m}", bufs=3)
                    if m % 2 == 0:
                        nc.scalar.activation(osb[:], ps_o0[m][:], AF.Copy)
                    else:
                        nc.vector.tensor_copy(out=osb[:], in_=ps_o0[m][:])
                    nc.sync.dma_start(
                        out=out_d[m * 128:(m + 1) * 128, 0:512],
                        in_=osb[:],
                    )

                # ---- remaining output projection blocks ----
                for jb in range(1, 4):
                    if wob[jb] is None:
                        wob[jb] = big.tile([128, 16, 512], BF16, tag="big",
                                           name=f"wob{jb}")
                        nc.sync.dma_start(out=wob[jb][:], in_=wo_d[jb])
                    for m in range(4):
                        ps_o = ps.tile([128, 512], F32, tag="ps",
                                       name=f"ps_o{jb}_{m}")
                        nc.tensor.matmul(
                            ps_o[:], ones_row[0:1, 0:128],
                            bo_sb[0:1, jb * 512:(jb + 1) * 512],
                            start=True, stop=False, skip_group_check=True,
                        )
                        for i in range(16):
                            nc.tensor.matmul(
                                ps_o[:], yt[:, i, m * 128:(m + 1) * 128],
                                wob[jb][:, i, :], start=False, stop=(i == 15),
                                skip_group_check=True,
                            )
                        osb = blkf.tile([128, 512], F32, tag="osb",
                                        name=f"osb{jb}_{m}", bufs=3)
                        if jb == 3:
                            nc.scalar.activation(
                                osb[:, 0:256], ps_o[:, 0:256], AF.Copy
                            )
                            nc.sync.dma_start(
                                out=out_d[m * 128:(m + 1) * 128,
                                          jb * 512:jb * 512 + 256],
                                in_=osb[:, 0:256],
                            )
                            nc.vector.tensor_copy(
                                out=osb[:, 256:512], in_=ps_o[:, 256:512]
                            )
                            nc.sync.dma_start(
                                out=out_d[m * 128:(m + 1) * 128,
                                          jb * 512 + 256:(jb + 1) * 512],
                                in_=osb[:, 256:512],
                            )
                        else:
                            if m % 2 == 0:
                                nc.scalar.activation(osb[:], ps_o[:], AF.Copy)
                            else:
                                nc.vector.tensor_copy(out=osb[:], in_=ps_o[:])
                            nc.sync.dma_start(
                                out=out_d[m * 128:(m + 1) * 128,
                                          jb * 512:(jb + 1) * 512],
                                in_=osb[:],
                            )

    nc.compile()
    return nc


_NC_CACHE = None
LAST_RES = None


def _get_nc():
    global _NC_CACHE
    if _NC_CACHE is None:
        _NC_CACHE = build_kernel()
    return _NC_CACHE


def _prep_shared(Wq, bq, Wk, bk, Wv, bv, ln_w, ln_b, Wo, bo):
    import ml_dtypes

    BF = ml_dtypes.bfloat16
    s = np.float32(SCALE)
    WqT = np.ascontiguousarray(Wq.T) * s  # [c, ch], scale folded into q
    # wq[j, p, i, cc] = WqT[i*128+p, j*128+cc]
    wq = np.ascontiguousarray(
        WqT.reshape(16, 128, 16, 128).transpose(2, 1, 0, 3).astype(BF)
    )
    WkT = np.ascontiguousarray(Wk.T)  # [2048, 512]
    wk = np.ascontiguousarray(
        WkT.reshape(16, 128, 4, 128).transpose(2, 1, 0, 3).astype(BF)
    )
    WvT = np.ascontiguousarray(Wv.T)  # [2048, 512]
    wv = np.ascontiguousarray(
        WvT.reshape(16, 128, KVC).transpose(1, 0, 2).astype(BF)
    )
    WoT = np.ascontiguousarray(Wo.T)  # [2048, 2048]
    wo = np.ascontiguousarray(
        WoT.reshape(16, 128, 4, 512).transpose(2, 1, 0, 3).astype(BF)
    )
    return {
        "wq": wq,
        "wk": wk,
        "wv": wv,
        "wo": wo,
        "bq": np.ascontiguousarray((bq * s).reshape(16, 128).T),
        "bk": np.ascontiguousarray(bk.reshape(4, 128).T),
        "bv": np.ascontiguousarray(bv.reshape(1, KVC)),
        "bo": np.ascontiguousarray(bo.reshape(1, C)),
        "lnw": np.ascontiguousarray(ln_w.reshape(16, 128).T),
        "lnb": np.ascontiguousarray(ln_b.reshape(16, 128).T),
        "onesb": np.ones((128, 1), BF),
        "onesr": np.ones((1, 512), np.float32),
    }


def _xt_full_tiled(x):
    # x [T, C] -> x^T tiled chunk-major [4, 128, 16, 512] bf16
    import ml_dtypes

    xT = np.ascontiguousarray(x.T)  # [C, T]
    return np.ascontiguousarray(
        xT.reshape(16, 128, 4, 512).transpose(2, 1, 0, 3).astype(ml_dtypes.bfloat16)
    )


def _xt_tiled(x):
    # x [R, C] -> x^T tiled [128, 16, R] bf16
    import ml_dtypes

    xT = np.ascontiguousarray(x.T)  # [C, R]
    return np.ascontiguousarray(
        xT.reshape(16, 128, R).transpose(1, 0, 2).astype(ml_dtypes.bfloat16)
    )


def kernel(
    query, key, value, Wq, bq, Wk, bk, Wv, bv, ln_w, ln_b, Wo, bo
):
    query = np.asarray(query, np.float32)
    key = np.asarray(key, np.float32)
    value = np.asarray(value, np.float32)

    nc = _get_nc()
    shared = _prep_shared(
        np.asarray(Wq, np.float32), np.asarray(bq, np.float32),
        np.asarray(Wk, np.float32), np.asarray(bk, np.float32),
        np.asarray(Wv, np.float32), np.asarray(bv, np.float32),
        np.asarray(ln_w, np.float32), np.asarray(ln_b, np.float32),
        np.asarray(Wo, np.float32), np.asarray(bo, np.float32),
    )

    xk_b = [_xt_full_tiled(key[b]) for b in range(B)]
    xv_b = [_xt_full_tiled(value[b]) for b in range(B)]

    in_maps = []
    for c in range(N_CORES):
        b = c // 4
        r0 = (c % 4) * R
        m = dict(shared)
        m["xq"] = _xt_tiled(query[b, r0:r0 + R, :])
        m["xk"] = xk_b[b]
        m["xv"] = xv_b[b]
        in_maps.append(m)

    res = run_bass_kernel_spmd(nc, in_maps, core_ids=list(range(N_CORES)))
    global LAST_RES
    LAST_RES = res

    out = np.empty((B, T, C), np.float32)
    for c in range(N_CORES):
        b = c // 4
        r0 = (c % 4) * R
        out[b, r0:r0 + R, :] = res.results[c]["out"]
    return out
